# revision 14
# baseline (speedup 1.0000x reference)
"""Trainium2 Bass kernel for nn_ContrastLoss (band-limited PSD contrastive loss).

Math notes (all exact identities, not approximations):
  - reference subtracts the per-window mean, but integer-frequency DFT bins
    23..102 are orthogonal to DC, so mean subtraction is a no-op on the band.
  - the band PSD is normalized per window (band / band.sum()), so the
    reference's 1/DELTA_T rfft scaling cancels; raw |X_f|^2 suffices.
  - rfft band bins are two real matmuls: X_f = seg @ cos_f, seg @ sin_f.

v2 (fp8): segments and DFT basis quantized to float8 e4m3 (validated
9.7e-5 scale-relative loss error vs 2e-2 budget — quantization noise is
per-window-random and cancels in the pairwise-MSE bilinear forms). This
halves HBM traffic (the binding resource) and enables DoubleRow matmuls
(256-deep contraction per instruction).

Device work per core (8-way channel sharding, 1024 windows/core):
  Input = w basis [128,1280]B + 4 seg pair blocks [128,2048]B, five DMAs
  split across both HWDGE queues (sync: pairs 0,2; scalar: w, pairs 1,3).
  Per 128-window chunk m: 4 accumulating DoubleRow fp8 matmuls vs the
  cos||sin basis -> PSUM [128win, 160] f32; DVE squares straight out of
  PSUM into bf16; the cos^2+sin^2 fold (GPSIMD chunks 0-5, DVE 6-7)
  writes two [128, 320] bf16 halves, DMA'd back independently so the
  first half overlaps the second half's compute.
Host: window gather + fp8 shard prep; row-sums, normalization and the
closed-form pairwise-MSE scalars in float64 (cheap: 8x1024x80 values).
"""

import sys

import numpy as np

if "/opt/trn_rl_repo" not in sys.path:
    sys.path.insert(0, "/opt/trn_rl_repo")

import ml_dtypes

B = 2
C = 256
T = 8192
K = 16
DT = 1024
NCORES = 8
CLOC = C // NCORES          # channels per core
SEGS = B * CLOC * K         # windows per core = 1024
F_LO, F_HI = 23, 103        # band bins [23, 102]
NF = F_HI - F_LO            # 80
NW = 2 * NF                 # 160 (cos || sin)
MCH = SEGS // 128           # 8 window chunks
JCH = DT // 256             # 4 double-row contraction blocks
N_TOT = C * K               # 4096 windows per video

FP8 = ml_dtypes.float8_e4m3fn


def _dft_basis():
    t = np.arange(DT, dtype=np.float64)
    f = np.arange(F_LO, F_HI, dtype=np.float64)
    ang = 2.0 * np.pi * np.outer(t, f) / DT
    w = np.concatenate([np.cos(ang), np.sin(ang)], axis=1)   # [DT, NW]
    # time t = 256j + 128i + p  ->  [p, j, i, f]
    return np.ascontiguousarray(
        w.reshape(JCH, 2, 128, NW).transpose(2, 0, 1, 3).astype(FP8)
    )


_W8 = _dft_basis()
_NC = None


def _build_nc():
    import concourse.mybir as mybir
    import concourse.tile as tile
    from concourse import bacc

    nc = bacc.Bacc(
        "TRN2",
        target_bir_lowering=False,
        debug=False,
        enable_asserts=True,
        num_devices=NCORES,
    )
    f32 = mybir.dt.float32
    bf16 = mybir.dt.bfloat16
    fp8 = mybir.dt.float8e4
    dr = mybir.MatmulPerfMode.DoubleRow
    segs_d = nc.dram_tensor(
        "segs", [MCH // 2, 128, 2 * DT], fp8, kind="ExternalInput"
    ).ap()
    w_d = nc.dram_tensor("w", [128, JCH * 2 * NW], fp8, kind="ExternalInput").ap()
    out_d = nc.dram_tensor("out", [128, MCH * NF], bf16, kind="ExternalOutput").ap()

    add = mybir.AluOpType.add
    mult = mybir.AluOpType.mult

    with tile.TileContext(nc) as tc:
        with (
            tc.tile_pool(name="segp", bufs=MCH // 2) as segp,
            tc.tile_pool(name="wp", bufs=1) as wp,
            tc.tile_pool(name="warmp", bufs=1) as warmp,
            tc.tile_pool(name="sqp", bufs=8) as sqp,
            tc.tile_pool(name="outp", bufs=2) as outp,
            tc.tile_pool(name="psum", bufs=8, space="PSUM") as psump,
        ):
            # PE warm-up: dummy matmuls with no input-data deps run during
            # the DMA fill, so the HAM clock gate is ramped when the real
            # matmuls arrive.
            scratch = warmp.tile([128, 512], bf16)
            nc.vector.memset(scratch[:], 0.0)
            warm_ps = psump.tile([128, 512], f32, tag="ps")
            for i in range(6):
                nc.tensor.matmul(
                    warm_ps[:, :512],
                    scratch[:, :128],
                    scratch[:, :512],
                    start=(i == 0),
                    stop=(i == 5),
                )

            # input stream: 5 DMAs split across both HWDGE queues so the
            # first matmul (needs w + pair0) unblocks earliest.
            w_t = wp.tile([128, JCH, 2, NW], fp8)
            seg_t = []
            for p in range(MCH // 2):
                st = segp.tile([128, 2, JCH, 2, 128], fp8, tag="seg")
                seg_t.append(st)
            # First-compute needs w + pair0: split across both rings
            # (sync's ring starts ~0.7us before scalar's, which carries
            # the ACT table load). Remaining bytes balanced so both rings
            # finish their inputs together.
            nc.sync.dma_start(seg_t[0][:], segs_d[0])
            nc.scalar.dma_start(w_t[:], w_d[:])
            nc.sync.dma_start(seg_t[1][:], segs_d[1])
            nc.scalar.dma_start(seg_t[2][:], segs_d[2])
            nc.sync.dma_start(seg_t[3][:], segs_d[3])
            sqf = mybir.ActivationFunctionType.Square

            # outA carries chunks 0-5 (issued once fold5 lands, overlaps the
            # tail); outB carries only chunks 6-7 so the last DMA is small
            # and gated by a short deterministic chain: fold6 on GPSIMD
            # (after ACT6), fold7 on DVE right behind its own square (no
            # cross-engine hop).
            outbuf_a = outp.tile([128, 6 * NF], bf16)
            outbuf_b = outp.tile([128, 2 * NF], bf16)
            for m in range(MCH):
                pr, c = divmod(m, 2)
                ps = psump.tile([128, NW], f32, tag="ps")
                for j in range(JCH):
                    nc.tensor.matmul(
                        ps[:],
                        seg_t[pr][:, c, j],
                        w_t[:, j],
                        start=(j == 0),
                        stop=(j == JCH - 1),
                        perf_mode=dr,
                    )
                # ScalarE ACT squares straight out of PSUM (single pass —
                # the BIR verifier forbids dual-PSUM-operand TensorTensor).
                # Chunk 7's square runs on DVE (copy+mult) so the last two
                # squares proceed in parallel instead of serializing on
                # the scalar queue.
                sq = sqp.tile([128, NW], bf16, tag="sq")
                if m < MCH - 1:
                    nc.scalar.activation(sq[:], ps[:], sqf)
                else:
                    x7 = sqp.tile([128, NW], f32, tag="x7")
                    nc.vector.tensor_copy(x7[:], ps[:])
                    nc.vector.tensor_tensor(out=sq[:], in0=x7[:], in1=x7[:], op=mult)
                if m < 6:
                    fout = outbuf_a[:, m * NF:(m + 1) * NF]
                else:
                    fout = outbuf_b[:, (m - 6) * NF:(m - 5) * NF]
                feng = nc.vector if m == MCH - 1 else nc.gpsimd
                feng.tensor_tensor(
                    out=fout, in0=sq[:, :NF], in1=sq[:, NF:], op=add
                )
                if m == MCH - 1:
                    nc.sync.dma_start(out_d[:, 6 * NF:], outbuf_b[:])
                elif m == 5:
                    nc.sync.dma_start(out_d[:, : 6 * NF], outbuf_a[:])

    nc.compile()
    return nc


def _get_nc():
    global _NC
    if _NC is None:
        _NC = _build_nc()
    return _NC


def _prep_in_maps(model_output, offsets):
    model_output = np.ascontiguousarray(model_output, dtype=np.float32)
    off = np.asarray(offsets, dtype=np.int64)
    sw = np.lib.stride_tricks.sliding_window_view(model_output, DT, axis=-1)
    bi = np.arange(B)[:, None, None]
    ci = np.arange(C)[None, :, None]
    seg = sw[bi, ci, off]                       # [B, C, K, DT] f32
    seg8 = seg.astype(FP8)
    in_maps = []
    for c in range(NCORES):
        sl = seg8[:, c * CLOC:(c + 1) * CLOC].reshape(SEGS, DT)
        # [pair, p, ((c2, j, i) blocks, s)] with time t = 256j + 128i + p
        arr = np.ascontiguousarray(
            sl.reshape(MCH // 2, 2, 128, JCH, 2, 128)  # [P, c2, s, j, i, p]
            .transpose(0, 5, 1, 3, 4, 2)               # [P, p, c2, j, i, s]
            .reshape(MCH // 2, 128, 2 * DT)
        )
        in_maps.append({"segs": arr, "w": _W8.reshape(128, JCH * 2 * NW)})
    return in_maps


def _finish(results):
    s = np.zeros((B, NF), dtype=np.float64)
    sq = np.zeros(B, dtype=np.float64)
    for c in range(NCORES):
        band8 = (
            results[c]["out"].astype(np.float64)
            .reshape(128, MCH, NF).transpose(1, 0, 2)   # [MCH, 128, NF]
        )
        rs = band8.sum(axis=-1)                         # [MCH, 128]
        q = (band8 * band8).sum(axis=-1)                # [MCH, 128]
        pn_sum = band8 / rs[..., None]
        for m in range(MCH):
            v = m // (MCH // B)
            s[v] += pn_sum[m].sum(axis=0)
            sq[v] += (q[m] / (rs[m] * rs[m])).sum()
    n = float(N_TOT)
    pos_per = (2.0 * n * sq - 2.0 * (s * s).sum(-1)) / NF / (n * n - n)
    pos = (pos_per[0] + pos_per[1]) / 2.0
    neg = -(n * sq[0] + n * sq[1] - 2.0 * float(np.dot(s[0], s[1]))) / NF / (n * n)
    return np.float32(pos + neg), np.float32(pos), np.float32(neg)


def kernel(model_output, offsets):
    from concourse.bass_utils import run_bass_kernel_spmd

    nc = _get_nc()
    in_maps = _prep_in_maps(model_output, offsets)
    res = run_bass_kernel_spmd(nc, in_maps, core_ids=list(range(NCORES)))
    return _finish(res.results)
